# revision 3
# baseline (speedup 1.0000x reference)
import os
import sys

sys.path.insert(0, "/opt/trn_rl_repo")

import numpy as np
import ml_dtypes
import bass_rust
from concourse import bass, mybir
from concourse.tile import TileContext
from concourse.vector_clock import ScopedClock
from concourse.bass_utils import run_bass_kernel_spmd

B, S, E, H = 4, 2048, 1024, 1024
NCORES = 8
NT = 8  # q-tiles per core, 128 rows each; local tile t -> global row-tile 2t+h
EC = E // 128
HC = H // 128
F32 = mybir.dt.float32
BF16 = mybir.dt.bfloat16
BF = ml_dtypes.bfloat16

LAST_RESULT = None


class PatchedTileContext(TileContext):
    """TileContext whose tail drain carries at most one sem wait.

    The walrus codegen in this container rejects a Drain with more than one
    sync wait ("Too many sync wait commands"); split the global-clock waits
    across a chain of drains on the same engine instead.
    """

    def _drain_and_barrier(self, tick_clock, wait_clock):
        drain_inst = self.nc.sync.drain()
        wait_clock.add_sem_waits(
            drain_inst.ins, ScopedClock({None: tick_clock.global_clock})
        )
        mi = drain_inst.ins
        waits = list(mi.sync_info.on_wait)
        ups = list(mi.sync_info.on_update)
        if len(waits) > 1:
            mi.sync_info = bass_rust.SyncInfo(on_wait=waits[:1], on_update=[])
            for i, w in enumerate(waits[1:]):
                d2 = self.nc.sync.drain()
                last = i == len(waits) - 2
                d2.ins.sync_info = bass_rust.SyncInfo(
                    on_wait=[w], on_update=ups if last else []
                )
        self.nc.all_engine_barrier()
        assert self.sems is not None
        popped = self.nc._tile_sem_poison_stack.pop()
        assert popped is self._sem_poison
        self.nc.clear_and_free_semaphores(list(self.sems.allocated().values()))
        self.nc.all_engine_barrier()


def _split_multi_waits(json_bytes):
    """Rewrite BIR so no instruction carries more than one sync wait."""
    import json as _json

    d = _json.loads(json_bytes)
    ctr = 0
    for f in d.get("functions", []):
        for blk in f.get("blocks", []):
            insts = blk.get("instructions", [])
            out = []
            for inst in insts:
                si = inst.get("sync_info") or {}
                ow = si.get("on_wait") or []
                if len(ow) > 1:
                    for w in ow[:-1]:
                        out.append(
                            {
                                "debug": inst.get("debug", 0),
                                "engine": inst["engine"],
                                "ins": [],
                                "name": f"wsplit_{ctr}",
                                "opcode": "NoOp",
                                "outs": [],
                                "sync_info": {"on_update": [], "on_wait": [w]},
                            }
                        )
                        ctr += 1
                    si = dict(si)
                    si["on_wait"] = [ow[-1]]
                    inst = dict(inst)
                    inst["sync_info"] = si
                out.append(inst)
            blk["instructions"] = out
    return _json.dumps(d).encode()


def _build_program():
    nc = bass.Bass("TRN2", target_bir_lowering=False, debug=False, num_devices=NCORES)
    orig_to_json_bytes = nc.to_json_bytes
    nc.to_json_bytes = lambda: _split_multi_waits(orig_to_json_bytes())

    xT = nc.dram_tensor("xT", [E, S], BF16, kind="ExternalInput")
    wqT = nc.dram_tensor("wqT", [E, H], BF16, kind="ExternalInput")
    wkT = nc.dram_tensor("wkT", [E, H], BF16, kind="ExternalInput")
    wvT = nc.dram_tensor("wvT", [E, H], BF16, kind="ExternalInput")
    bqd = nc.dram_tensor("bqd", [128, HC], F32, kind="ExternalInput")
    bkd = nc.dram_tensor("bkd", [128, HC], F32, kind="ExternalInput")
    bvd = nc.dram_tensor("bvd", [H], F32, kind="ExternalInput")
    maskd = nc.dram_tensor("maskd", [128, 256], F32, kind="ExternalInput")
    onesd = nc.dram_tensor("onesd", [128, 8], BF16, kind="ExternalInput")
    out = nc.dram_tensor("out", [NT, 128, H], F32, kind="ExternalOutput")

    # psum slot tags: 2 + 4 + 2 slots of [128,512]f32 = exactly 8 banks
    KTAGS = ["pk", "pk", "po", "po", "po", "po", "pol", "pol"]
    TAGBUFS = {"pk": 2, "po": 4, "pol": 2}

    with PatchedTileContext(nc) as tc:
        with (
            tc.tile_pool(name="const", bufs=1) as constp,
            tc.tile_pool(name="resx", bufs=1) as xp,
            tc.tile_pool(name="resk", bufs=1) as kp,
            tc.tile_pool(name="resv", bufs=1) as vvp,
            tc.tile_pool(name="resq", bufs=1) as qp,
            tc.tile_pool(name="wts", bufs=1) as wp,
            tc.tile_pool(name="ptp", bufs=3) as ptp,
            tc.tile_pool(name="otp", bufs=2) as otp,
            tc.tile_pool(name="stat", bufs=4) as statp,
            tc.tile_pool(name="pp", bufs=1, space="PSUM") as pp,
        ):
            bk_t = constp.tile([128, HC], F32, tag="bkt")
            bq_t = constp.tile([128, HC], F32, tag="bqt")
            bv_bc = constp.tile([128, H], F32, tag="bvb")
            mask_sb = constp.tile([128, 256], F32, tag="msk")
            ones_sb = constp.tile([128, 8], BF16, tag="ones")
            warm = constp.tile([128, 1], F32, tag="warm")

            xsb = xp.tile([128, EC, S], BF16, tag="x")
            kt = kp.tile([128, HC, S], BF16, tag="kt")
            vsb = vvp.tile([128, 16, H], BF16, tag="v")
            qt = qp.tile([128, HC, NT * 128], BF16, tag="qt")
            wk = wp.tile([128, EC, H], BF16, tag="wk")
            wv = wp.tile([128, EC, H], BF16, tag="wv")
            wq = wp.tile([128, EC, H], BF16, tag="wq")

            nc.gpsimd.dma_start(out=bk_t, in_=bkd[:, :])
            nc.gpsimd.dma_start(out=bq_t, in_=bqd[:, :])
            nc.gpsimd.dma_start(out=mask_sb, in_=maskd[:, :])
            nc.gpsimd.dma_start(out=ones_sb, in_=onesd[:, :])
            nc.gpsimd.dma_start(out=bv_bc, in_=bvd[:].partition_broadcast(128))

            # All bulk input DMA on the sync queue (hardware DGE — engine-
            # issued DMAs go through slow software descriptor generation).
            # K-phase inputs first, interleaved per e-chunk so the PE can
            # start early; then x key-blocks 1-3, then wv, wq.
            for e in range(EC):
                nc.sync.dma_start(out=wk[:, e, :], in_=wkT[e * 128 : (e + 1) * 128, :])
                nc.sync.dma_start(
                    out=xsb[:, e, 0:512], in_=xT[e * 128 : (e + 1) * 128, 0:512]
                )
            for blk in range(1, 4):
                for e in range(EC):
                    nc.sync.dma_start(
                        out=xsb[:, e, blk * 512 : (blk + 1) * 512],
                        in_=xT[e * 128 : (e + 1) * 128, blk * 512 : (blk + 1) * 512],
                    )
            for e in range(EC):
                nc.sync.dma_start(out=wv[:, e, :], in_=wvT[e * 128 : (e + 1) * 128, :])
            for e in range(EC):
                nc.sync.dma_start(out=wq[:, e, :], in_=wqT[e * 128 : (e + 1) * 128, :])

            # exp activation-table warm-up (depends only on bk_t DMA)
            nc.scalar.activation(
                warm, bk_t[:, 0:1], mybir.ActivationFunctionType.Exp
            )

            def psum_tile(tag, name):
                return pp.tile(
                    [128, 512], F32, tag=tag, bufs=TAGBUFS[tag], name=name
                )

            # ---- K projection: kt[hd, s] = sum_e wk[e, hd] * x[e, s] + bk ----
            # slice 0 e-outer (DMA streams in e order); slices 1-3 hc-outer so
            # drains trickle and bank reuse never stalls PE.
            for sl in range(4):
                ks = slice(sl * 512, (sl + 1) * 512)
                if sl == 0:
                    ps = [psum_tile(KTAGS[hc], f"psk0_{hc}") for hc in range(HC)]
                    for e in range(EC):
                        for hc in range(HC):
                            nc.tensor.matmul(
                                ps[hc],
                                lhsT=wk[:, e, hc * 128 : (hc + 1) * 128],
                                rhs=xsb[:, e, ks],
                                start=(e == 0),
                                stop=(e == EC - 1),
                            )
                    for hc in range(HC):
                        nc.vector.tensor_scalar_add(
                            kt[:, hc, ks], ps[hc], bk_t[:, hc : hc + 1]
                        )
                else:
                    for hc in range(HC):
                        psg = psum_tile(KTAGS[hc], f"psk{sl}_{hc}")
                        for e in range(EC):
                            nc.tensor.matmul(
                                psg,
                                lhsT=wk[:, e, hc * 128 : (hc + 1) * 128],
                                rhs=xsb[:, e, ks],
                                start=(e == 0),
                                stop=(e == EC - 1),
                            )
                        nc.vector.tensor_scalar_add(
                            kt[:, hc, ks], psg, bk_t[:, hc : hc + 1]
                        )

            # ---- V projection: v[s, h] = sum_e x[e, s] * wv[e, h] + bv ----
            for sc in range(16):
                tg = "pk" if sc % 2 == 0 else "po"
                pv = [psum_tile(tg, f"psv{sc}_{hh}") for hh in range(2)]
                for e in range(EC):
                    for hh in range(2):
                        nc.tensor.matmul(
                            pv[hh],
                            lhsT=xsb[:, e, sc * 128 : (sc + 1) * 128],
                            rhs=wv[:, e, hh * 512 : (hh + 1) * 512],
                            start=(e == 0),
                            stop=(e == EC - 1),
                        )
                for hh in range(2):
                    hs = slice(hh * 512, (hh + 1) * 512)
                    nc.vector.tensor_add(vsb[:, sc, hs], pv[hh], bv_bc[:, hs])

            # ---- Q projection: qt[hd, q] over the core's 8 q-tiles, which sit
            # at even 128-col slots of (permuted) xT.
            qi = 0
            for hc in range(HC):
                for hh in range(2):
                    pq = psum_tile("pk" if qi % 2 == 0 else "po", f"psq{hc}_{hh}")
                    qi += 1
                    for e in range(EC):
                        rhs = xsb[:, e, hh * 1024 : (hh + 1) * 1024].rearrange(
                            "p (t q) -> p t q", q=256
                        )[:, :, 0:128]
                        nc.tensor.matmul(
                            pq,
                            lhsT=wq[:, e, hc * 128 : (hc + 1) * 128],
                            rhs=rhs,
                            start=(e == 0),
                            stop=(e == EC - 1),
                        )
                    nc.vector.tensor_scalar_add(
                        qt[:, hc, hh * 512 : (hh + 1) * 512], pq, bq_t[:, hc : hc + 1]
                    )

            # ---- attention: S^T chunks -> exp -> AV, software-pipelined ----
            units = []
            for t in range(NT):
                nkc = 2 * t + 2
                cs = list(range(nkc))
                gs = [cs[i : i + 4] for i in range(0, nkc, 4)]
                for gi, g in enumerate(gs):
                    units.append((t, gi, g, gi == len(gs) - 1))

            po_of = {}

            def issue_scores(u):
                t, gi, g, last = u
                w = len(g) * 128
                st = pp.tile(
                    [128, w], F32, tag="pk", bufs=2, name=f"st{t}_{gi}"
                )
                for ci, c in enumerate(g):
                    for hc in range(HC):
                        nc.tensor.matmul(
                            st[:, ci * 128 : (ci + 1) * 128],
                            lhsT=kt[:, hc, c * 128 : (c + 1) * 128],
                            rhs=qt[:, hc, t * 128 : (t + 1) * 128],
                            start=(hc == 0),
                            stop=(hc == HC - 1),
                        )
                if last:
                    nc.vector.tensor_add(
                        st[:, w - 256 : w], st[:, w - 256 : w], mask_sb
                    )
                pt = ptp.tile([128, len(g), 128], BF16, tag="pt", name=f"pt{t}_{gi}")
                nc.scalar.activation(
                    pt,
                    st.rearrange("p (c q) -> p c q", q=128),
                    mybir.ActivationFunctionType.Exp,
                )
                return pt

            def issue_av(u, pt):
                t, gi, g, last = u
                nkc = 2 * t + 2
                if gi == 0:
                    po0 = psum_tile("po", f"po0_{t}")
                    po1 = psum_tile("po", f"po1_{t}")
                    pol = pp.tile([128, 8], F32, tag="pol", bufs=2, name=f"pol_{t}")
                    po_of[t] = (po0, po1, pol)
                po0, po1, pol = po_of[t]
                for ci, c in enumerate(g):
                    lt = pt[:, ci, :]
                    fl = (c == 0, c == nkc - 1)
                    nc.tensor.matmul(
                        po0, lhsT=lt, rhs=vsb[:, c, 0:512], start=fl[0], stop=fl[1]
                    )
                    nc.tensor.matmul(
                        po1, lhsT=lt, rhs=vsb[:, c, 512:1024], start=fl[0], stop=fl[1]
                    )
                    nc.tensor.matmul(
                        pol, lhsT=lt, rhs=ones_sb, start=fl[0], stop=fl[1]
                    )
                if last:
                    rl = statp.tile([128, 1], F32, tag="rl", name=f"rl{t}")
                    nc.vector.reciprocal(rl, pol[:, 0:1])
                    ot = otp.tile([128, H], F32, tag="ot", name=f"ot{t}")
                    # halves drain on different engines so the tail is short
                    nc.vector.tensor_scalar_mul(ot[:, 0:512], po0, rl)
                    nc.scalar.activation(
                        ot[:, 512:1024],
                        po1,
                        mybir.ActivationFunctionType.Copy,
                        scale=rl,
                    )
                    nc.sync.dma_start(out=out[t, :, 0:512], in_=ot[:, 0:512])
                    nc.sync.dma_start(out=out[t, :, 512:1024], in_=ot[:, 512:1024])

            prev = None
            for u in units:
                pt_u = issue_scores(u)
                if prev is not None:
                    issue_av(*prev)
                prev = (u, pt_u)
            issue_av(*prev)

    return nc


def kernel(inputs, Wq, bq, Wk, bk, Wv, bv):
    global LAST_RESULT
    inputs = np.ascontiguousarray(inputs, dtype=np.float32)
    scale = 1.0 / np.sqrt(np.float32(E))

    wqT = np.ascontiguousarray((Wq.T.astype(np.float32) * scale).astype(BF))
    wkT = np.ascontiguousarray(Wk.T.astype(np.float32).astype(BF))
    wvT = np.ascontiguousarray(Wv.T.astype(np.float32).astype(BF))
    bqs = np.ascontiguousarray((bq.astype(np.float32) * scale).reshape(HC, 128).T)
    bk2 = np.ascontiguousarray(bk.astype(np.float32).reshape(HC, 128).T)
    bv = np.ascontiguousarray(bv, dtype=np.float32)
    ones8 = np.ones((128, 8), dtype=BF)

    kk = np.arange(128)[:, None]
    qq = np.arange(128)[None, :]
    tri = np.where(qq >= kk, 0.0, -1e30).astype(np.float32)
    mask_h = [
        np.concatenate([tri, np.full((128, 128), -1e30, np.float32)], axis=1),
        np.concatenate([tri, np.zeros((128, 128), np.float32)], axis=1),
    ]
    # h=1 cores see key blocks pair-swapped so q-tiles sit at even slots
    perm1 = np.concatenate([np.arange(128) + 128 * (j ^ 1) for j in range(16)])

    in_maps = []
    for c in range(NCORES):
        b, h = c // 2, c % 2
        xTb = inputs[b].T  # [E, S] f32
        if h == 1:
            xTb = xTb[:, perm1]
        in_maps.append(
            {
                "xT": np.ascontiguousarray(xTb.astype(BF)),
                "wqT": wqT,
                "wkT": wkT,
                "wvT": wvT,
                "bqd": bqs,
                "bkd": bk2,
                "bvd": bv,
                "maskd": mask_h[h],
                "onesd": ones8,
            }
        )

    nc = _build_program()
    res = None
    last_err = None
    for attempt in range(3):
        try:
            res = run_bass_kernel_spmd(nc, in_maps, list(range(NCORES)))
            break
        except Exception as e:  # transient NRT device wedge; retry
            last_err = e
            import time as _time

            _time.sleep(2.0)
    if res is None:
        raise last_err
    LAST_RESULT = res

    out = np.empty((B, S, H), dtype=np.float32)
    for c in range(NCORES):
        b, h = c // 2, c % 2
        o = res.results[c]["out"]  # [NT, 128, H] f32
        for t in range(NT):
            r = 2 * t + h
            out[b, r * 128 : (r + 1) * 128, :] = o[t]
    return out


# revision 4
# speedup vs baseline: 1.1627x; 1.1627x over previous
import os
import sys

sys.path.insert(0, "/opt/trn_rl_repo")

import numpy as np
import ml_dtypes
import bass_rust
from concourse import bass, mybir
from concourse.tile import TileContext
from concourse.vector_clock import ScopedClock
from concourse.bass_utils import run_bass_kernel_spmd

B, S, E, H = 4, 2048, 1024, 1024
NCORES = 8
NT = 8  # q-tiles per core, 128 rows each; local tile t -> global row-tile 2t+h
EC = E // 128
HC = H // 128
F32 = mybir.dt.float32
BF16 = mybir.dt.bfloat16
BF = ml_dtypes.bfloat16

LAST_RESULT = None


class PatchedTileContext(TileContext):
    """TileContext whose tail drain carries at most one sem wait.

    The walrus codegen in this container rejects a Drain with more than one
    sync wait ("Too many sync wait commands"); split the global-clock waits
    across a chain of drains on the same engine instead.
    """

    def _drain_and_barrier(self, tick_clock, wait_clock):
        drain_inst = self.nc.sync.drain()
        wait_clock.add_sem_waits(
            drain_inst.ins, ScopedClock({None: tick_clock.global_clock})
        )
        mi = drain_inst.ins
        waits = list(mi.sync_info.on_wait)
        ups = list(mi.sync_info.on_update)
        if len(waits) > 1:
            mi.sync_info = bass_rust.SyncInfo(on_wait=waits[:1], on_update=[])
            for i, w in enumerate(waits[1:]):
                d2 = self.nc.sync.drain()
                last = i == len(waits) - 2
                d2.ins.sync_info = bass_rust.SyncInfo(
                    on_wait=[w], on_update=ups if last else []
                )
        self.nc.all_engine_barrier()
        assert self.sems is not None
        popped = self.nc._tile_sem_poison_stack.pop()
        assert popped is self._sem_poison
        self.nc.clear_and_free_semaphores(list(self.sems.allocated().values()))
        self.nc.all_engine_barrier()


def _split_multi_waits(json_bytes):
    """Rewrite BIR so no instruction carries more than one sync wait."""
    import json as _json

    d = _json.loads(json_bytes)
    ctr = 0
    for f in d.get("functions", []):
        for blk in f.get("blocks", []):
            insts = blk.get("instructions", [])
            out = []
            for inst in insts:
                si = inst.get("sync_info") or {}
                ow = si.get("on_wait") or []
                if len(ow) > 1:
                    for w in ow[:-1]:
                        out.append(
                            {
                                "debug": inst.get("debug", 0),
                                "engine": inst["engine"],
                                "ins": [],
                                "name": f"wsplit_{ctr}",
                                "opcode": "NoOp",
                                "outs": [],
                                "sync_info": {"on_update": [], "on_wait": [w]},
                            }
                        )
                        ctr += 1
                    si = dict(si)
                    si["on_wait"] = [ow[-1]]
                    inst = dict(inst)
                    inst["sync_info"] = si
                out.append(inst)
            blk["instructions"] = out
    return _json.dumps(d).encode()


def _build_program():
    nc = bass.Bass("TRN2", target_bir_lowering=False, debug=False, num_devices=NCORES)
    orig_to_json_bytes = nc.to_json_bytes
    nc.to_json_bytes = lambda: _split_multi_waits(orig_to_json_bytes())

    xT = nc.dram_tensor("xT", [E, S], BF16, kind="ExternalInput")
    wqT = nc.dram_tensor("wqT", [E, H], BF16, kind="ExternalInput")
    wkT = nc.dram_tensor("wkT", [E, H], BF16, kind="ExternalInput")
    wvT = nc.dram_tensor("wvT", [E, H], BF16, kind="ExternalInput")
    bqd = nc.dram_tensor("bqd", [128, HC], F32, kind="ExternalInput")
    bkd = nc.dram_tensor("bkd", [128, HC], F32, kind="ExternalInput")
    bvd = nc.dram_tensor("bvd", [H], F32, kind="ExternalInput")
    maskd = nc.dram_tensor("maskd", [128, 256], F32, kind="ExternalInput")
    onesd = nc.dram_tensor("onesd", [128, 8], BF16, kind="ExternalInput")
    out = nc.dram_tensor("out", [NT, 128, H], F32, kind="ExternalOutput")

    # psum slot tags: 2 + 4 + 2 slots of [128,512]f32 = exactly 8 banks
    KTAGS = ["pk", "pk", "po", "po", "po", "po", "pol", "pol"]
    TAGBUFS = {"pk": 2, "po": 4, "pol": 2}

    with PatchedTileContext(nc) as tc:
        with (
            tc.tile_pool(name="const", bufs=1) as constp,
            tc.tile_pool(name="resx", bufs=1) as xp,
            tc.tile_pool(name="resk", bufs=1) as kp,
            tc.tile_pool(name="resv", bufs=1) as vvp,
            tc.tile_pool(name="resq", bufs=1) as qp,
            tc.tile_pool(name="wts", bufs=1) as wp,
            tc.tile_pool(name="ptp", bufs=3) as ptp,
            tc.tile_pool(name="otp", bufs=2) as otp,
            tc.tile_pool(name="stat", bufs=4) as statp,
            tc.tile_pool(name="pp", bufs=1, space="PSUM") as pp,
        ):
            bk_t = constp.tile([128, HC], F32, tag="bkt")
            bq_t = constp.tile([128, HC], F32, tag="bqt")
            bv_bc = constp.tile([128, H], F32, tag="bvb")
            mask_sb = constp.tile([128, 256], F32, tag="msk")
            ones_sb = constp.tile([128, 8], BF16, tag="ones")
            warm = constp.tile([128, 1], F32, tag="warm")

            xsb = xp.tile([128, EC, S], BF16, tag="x")
            kt = kp.tile([128, HC, S], BF16, tag="kt")
            vsb = vvp.tile([128, 16, H], BF16, tag="v")
            qt = qp.tile([128, HC, NT * 128], BF16, tag="qt")
            wk = wp.tile([128, EC, H], BF16, tag="wk")
            wv = wp.tile([128, EC, H], BF16, tag="wv")
            wq = wp.tile([128, EC, H], BF16, tag="wq")

            nc.gpsimd.dma_start(out=bk_t, in_=bkd[:, :])
            nc.gpsimd.dma_start(out=bq_t, in_=bqd[:, :])
            nc.gpsimd.dma_start(out=mask_sb, in_=maskd[:, :])
            nc.gpsimd.dma_start(out=bv_bc, in_=bvd[:].partition_broadcast(128))

            # tiny first transfer primes the sync DMA queue so the K-phase
            # stream doesn't pay the ~1.7us first-descriptor latency
            nc.sync.dma_start(out=ones_sb, in_=onesd[:, :])

            # All bulk input DMA on the sync queue (hardware DGE — engine-
            # issued DMAs go through slow software descriptor generation).
            # K-phase inputs first, interleaved per e-chunk so the PE can
            # start early; then x key-blocks 1-3, then wv, wq.
            for e in range(EC):
                nc.sync.dma_start(out=wk[:, e, :], in_=wkT[e * 128 : (e + 1) * 128, :])
                nc.sync.dma_start(
                    out=xsb[:, e, 0:512], in_=xT[e * 128 : (e + 1) * 128, 0:512]
                )
            for blk in range(1, 4):
                for e in range(EC):
                    nc.sync.dma_start(
                        out=xsb[:, e, blk * 512 : (blk + 1) * 512],
                        in_=xT[e * 128 : (e + 1) * 128, blk * 512 : (blk + 1) * 512],
                    )
            for e in range(EC):
                nc.sync.dma_start(out=wv[:, e, :], in_=wvT[e * 128 : (e + 1) * 128, :])
            for e in range(EC):
                nc.sync.dma_start(out=wq[:, e, :], in_=wqT[e * 128 : (e + 1) * 128, :])

            # exp activation-table warm-up (depends only on bk_t DMA)
            nc.scalar.activation(
                warm, bk_t[:, 0:1], mybir.ActivationFunctionType.Exp
            )

            def psum_tile(tag, name):
                return pp.tile(
                    [128, 512], F32, tag=tag, bufs=TAGBUFS[tag], name=name
                )

            # ---- K projection: kt[hd, s] = sum_e wk[e, hd] * x[e, s] + bk ----
            # slice 0 e-outer (DMA streams in e order); slices 1-3 hc-outer so
            # drains trickle and bank reuse never stalls PE.
            for sl in range(4):
                ks = slice(sl * 512, (sl + 1) * 512)
                if sl == 0:
                    ps = [psum_tile(KTAGS[hc], f"psk0_{hc}") for hc in range(HC)]
                    for e in range(EC):
                        for hc in range(HC):
                            nc.tensor.matmul(
                                ps[hc],
                                lhsT=wk[:, e, hc * 128 : (hc + 1) * 128],
                                rhs=xsb[:, e, ks],
                                start=(e == 0),
                                stop=(e == EC - 1),
                            )
                    for hc in range(HC):
                        nc.vector.tensor_scalar_add(
                            kt[:, hc, ks], ps[hc], bk_t[:, hc : hc + 1]
                        )
                else:
                    for hc in range(HC):
                        psg = psum_tile(KTAGS[hc], f"psk{sl}_{hc}")
                        for e in range(EC):
                            nc.tensor.matmul(
                                psg,
                                lhsT=wk[:, e, hc * 128 : (hc + 1) * 128],
                                rhs=xsb[:, e, ks],
                                start=(e == 0),
                                stop=(e == EC - 1),
                            )
                        nc.vector.tensor_scalar_add(
                            kt[:, hc, ks], psg, bk_t[:, hc : hc + 1]
                        )

            # ---- V projection: v[s, h] = sum_e x[e, s] * wv[e, h] + bv ----
            for sc in range(16):
                tg = "pk" if sc % 2 == 0 else "po"
                pv = [psum_tile(tg, f"psv{sc}_{hh}") for hh in range(2)]
                for e in range(EC):
                    for hh in range(2):
                        nc.tensor.matmul(
                            pv[hh],
                            lhsT=xsb[:, e, sc * 128 : (sc + 1) * 128],
                            rhs=wv[:, e, hh * 512 : (hh + 1) * 512],
                            start=(e == 0),
                            stop=(e == EC - 1),
                        )
                for hh in range(2):
                    hs = slice(hh * 512, (hh + 1) * 512)
                    nc.vector.tensor_add(vsb[:, sc, hs], pv[hh], bv_bc[:, hs])

            # ---- Q projection: qt[hd, q] over the core's 8 q-tiles, which sit
            # at even 128-col slots of (permuted) xT.
            qi = 0
            for hc in range(HC):
                for hh in range(2):
                    pq = psum_tile("pk" if qi % 2 == 0 else "po", f"psq{hc}_{hh}")
                    qi += 1
                    for e in range(EC):
                        rhs = xsb[:, e, hh * 1024 : (hh + 1) * 1024].rearrange(
                            "p (t q) -> p t q", q=256
                        )[:, :, 0:128]
                        nc.tensor.matmul(
                            pq,
                            lhsT=wq[:, e, hc * 128 : (hc + 1) * 128],
                            rhs=rhs,
                            start=(e == 0),
                            stop=(e == EC - 1),
                        )
                    nc.vector.tensor_scalar_add(
                        qt[:, hc, hh * 512 : (hh + 1) * 512], pq, bq_t[:, hc : hc + 1]
                    )

            # ---- attention: S^T chunks -> exp -> AV, software-pipelined ----
            units = []
            for t in range(NT):
                nkc = 2 * t + 2
                cs = list(range(nkc))
                gs = [cs[i : i + 4] for i in range(0, nkc, 4)]
                for gi, g in enumerate(gs):
                    units.append((t, gi, g, gi == len(gs) - 1))

            po_of = {}

            def issue_scores(u):
                t, gi, g, last = u
                w = len(g) * 128
                st = pp.tile(
                    [128, w], F32, tag="pk", bufs=2, name=f"st{t}_{gi}"
                )
                for ci, c in enumerate(g):
                    for hc in range(HC):
                        nc.tensor.matmul(
                            st[:, ci * 128 : (ci + 1) * 128],
                            lhsT=kt[:, hc, c * 128 : (c + 1) * 128],
                            rhs=qt[:, hc, t * 128 : (t + 1) * 128],
                            start=(hc == 0),
                            stop=(hc == HC - 1),
                        )
                if last:
                    nc.vector.tensor_add(
                        st[:, w - 256 : w], st[:, w - 256 : w], mask_sb
                    )
                pt = ptp.tile([128, len(g), 128], BF16, tag="pt", name=f"pt{t}_{gi}")
                nc.scalar.activation(
                    pt,
                    st.rearrange("p (c q) -> p c q", q=128),
                    mybir.ActivationFunctionType.Exp,
                )
                return pt

            def issue_av(u, pt):
                t, gi, g, last = u
                nkc = 2 * t + 2
                if gi == 0:
                    po0 = psum_tile("po", f"po0_{t}")
                    po1 = psum_tile("po", f"po1_{t}")
                    pol = pp.tile([128, 8], F32, tag="pol", bufs=2, name=f"pol_{t}")
                    po_of[t] = (po0, po1, pol)
                po0, po1, pol = po_of[t]
                if last:
                    # target-major order: pol stops first, then po0, then po1,
                    # so recip/scale0 overlap the remaining AV matmuls (the
                    # engine semaphore is cumulative, so earlier stop = lower
                    # wait threshold for the drain ops)
                    for tgt, rhs_of in (
                        (pol, lambda c: ones_sb),
                        (po0, lambda c: vsb[:, c, 0:512]),
                        (po1, lambda c: vsb[:, c, 512:1024]),
                    ):
                        for ci, c in enumerate(g):
                            nc.tensor.matmul(
                                tgt,
                                lhsT=pt[:, ci, :],
                                rhs=rhs_of(c),
                                start=(c == 0),
                                stop=(c == nkc - 1),
                            )
                else:
                    for ci, c in enumerate(g):
                        lt = pt[:, ci, :]
                        fl = (c == 0, c == nkc - 1)
                        nc.tensor.matmul(
                            po0, lhsT=lt, rhs=vsb[:, c, 0:512], start=fl[0], stop=fl[1]
                        )
                        nc.tensor.matmul(
                            po1, lhsT=lt, rhs=vsb[:, c, 512:1024], start=fl[0], stop=fl[1]
                        )
                        nc.tensor.matmul(
                            pol, lhsT=lt, rhs=ones_sb, start=fl[0], stop=fl[1]
                        )
                if last:
                    rl = statp.tile([128, 1], F32, tag="rl", name=f"rl{t}")
                    nc.vector.reciprocal(rl, pol[:, 0:1])
                    ot = otp.tile([128, H], F32, tag="ot", name=f"ot{t}")
                    # halves drain on different engines so the tail is short
                    nc.vector.tensor_scalar_mul(ot[:, 0:512], po0, rl)
                    nc.scalar.activation(
                        ot[:, 512:1024],
                        po1,
                        mybir.ActivationFunctionType.Copy,
                        scale=rl,
                    )
                    nc.sync.dma_start(out=out[t, :, 0:512], in_=ot[:, 0:512])
                    nc.sync.dma_start(out=out[t, :, 512:1024], in_=ot[:, 512:1024])

            prev = None
            for u in units:
                pt_u = issue_scores(u)
                if prev is not None:
                    issue_av(*prev)
                prev = (u, pt_u)
            issue_av(*prev)

    return nc


def kernel(inputs, Wq, bq, Wk, bk, Wv, bv):
    global LAST_RESULT
    inputs = np.ascontiguousarray(inputs, dtype=np.float32)
    scale = 1.0 / np.sqrt(np.float32(E))

    wqT = np.ascontiguousarray((Wq.T.astype(np.float32) * scale).astype(BF))
    wkT = np.ascontiguousarray(Wk.T.astype(np.float32).astype(BF))
    wvT = np.ascontiguousarray(Wv.T.astype(np.float32).astype(BF))
    bqs = np.ascontiguousarray((bq.astype(np.float32) * scale).reshape(HC, 128).T)
    bk2 = np.ascontiguousarray(bk.astype(np.float32).reshape(HC, 128).T)
    bv = np.ascontiguousarray(bv, dtype=np.float32)
    ones8 = np.ones((128, 8), dtype=BF)

    kk = np.arange(128)[:, None]
    qq = np.arange(128)[None, :]
    tri = np.where(qq >= kk, 0.0, -1e30).astype(np.float32)
    mask_h = [
        np.concatenate([tri, np.full((128, 128), -1e30, np.float32)], axis=1),
        np.concatenate([tri, np.zeros((128, 128), np.float32)], axis=1),
    ]
    # h=1 cores see key blocks pair-swapped so q-tiles sit at even slots
    perm1 = np.concatenate([np.arange(128) + 128 * (j ^ 1) for j in range(16)])

    in_maps = []
    for c in range(NCORES):
        b, h = c // 2, c % 2
        xTb = inputs[b].T  # [E, S] f32
        if h == 1:
            xTb = xTb[:, perm1]
        in_maps.append(
            {
                "xT": np.ascontiguousarray(xTb.astype(BF)),
                "wqT": wqT,
                "wkT": wkT,
                "wvT": wvT,
                "bqd": bqs,
                "bkd": bk2,
                "bvd": bv,
                "maskd": mask_h[h],
                "onesd": ones8,
            }
        )

    nc = _build_program()
    res = None
    last_err = None
    for attempt in range(3):
        try:
            res = run_bass_kernel_spmd(nc, in_maps, list(range(NCORES)))
            break
        except Exception as e:  # transient NRT device wedge; retry
            last_err = e
            import time as _time

            _time.sleep(2.0)
    if res is None:
        raise last_err
    LAST_RESULT = res

    out = np.empty((B, S, H), dtype=np.float32)
    for c in range(NCORES):
        b, h = c // 2, c % 2
        o = res.results[c]["out"]  # [NT, 128, H] f32
        for t in range(NT):
            r = 2 * t + h
            out[b, r * 128 : (r + 1) * 128, :] = o[t]
    return out
